# revision 7
# baseline (speedup 1.0000x reference)
"""Trainium2 Bass kernel for nn_Local_APro: affinity-based local propagation.

Reference computation (per image):
  F = img + 10
  aff_k = exp(-||F(p+delta_k) - F(p)||^2 / zeta^2)   (5x5 window, zero pad)
  sumz  = sum_k aff_k
  x0    = feat * mask
  repeat 20x:  x <- mask * (sum_k aff_k * x(p+delta_k)) / sumz

Sharding: 8 shards = 4 images x 2 height-halves. Each shard computes an
extended region of 232 rows (192 output + 40-row halo = 20 iters * 2 rows)
so there is ZERO cross-core communication; contaminated halo rows are
discarded on the host at gather time.

On-chip layout: partition dim = image rows; the 232 ext rows are two
116-row "panels" stored as the middle free dim of [128, 2, 388] tiles
(panel bases 0 and 116). Column shifts are free-axis byte offsets. Row
shifts cannot be partition-offset operand reads (compute engines must
start at partition-quadrant boundaries), so we keep 4 row-shifted copies
of x, refreshed each iteration by SBUF->SBUF DMA (DMA has no quadrant
constraint and is otherwise idle).

The +10 shift never happens on device: it cancels inside every img
difference, except against zero padding - so the host fills pad cells
with -10 (raw-image space), making (pad - center) == (0 - (center+10)).
"""

import os
import sys

import numpy as np

_REPO = "/opt/trn_rl_repo"
try:
    import concourse.bass  # noqa: F401
except Exception:
    if os.path.isdir(_REPO) and _REPO not in sys.path:
        sys.path.insert(0, _REPO)

import concourse.bacc as bacc
import concourse.mybir as mybir
import concourse.tile as tile
from concourse.bass_utils import run_bass_kernel_spmd

K = 5
ZETA = 0.15
NUM_ITER = 20
B, C, H, W = 4, 3, 384, 384
HALF = 192            # kept output rows per shard
HALO = 2 * NUM_ITER   # 40 contaminated rows next to the cut
EXT = HALF + HALO     # 232 rows computed per shard
PB = 236              # padded slab rows (2 + EXT + 2)
WP = W + 4            # padded width 388
PANEL = 116           # rows per panel (2 panels)
F32 = mybir.dt.float32

DRS = (-2, -1, 1, 2)  # row shifts needing shifted x copies (dr=0 is x itself)

# per-iteration tap order: a few dr=0 taps first (ready immediately after the
# previous iteration's last op), then +-2, +-1 (so each shifted tile's readers
# finish early, releasing it for the next iteration's DMA refresh), then the
# remaining dr=0 taps.
_dr0 = [(0, dw) for dw in (-2, -1, 1, 2)]
_TAPS = (_dr0[:2]
         + [(dr, dw) for dr in (2, -2, 1, -1) for dw in (-2, -1, 0, 1, 2)]
         + _dr0[2:])

_CACHE: dict = {}


def _body(tc, img_d, feat_d, mask_d, out_d):
    nc = tc.nc
    Exp = mybir.ActivationFunctionType.Exp
    NZ = -1.0 / (ZETA * ZETA)

    with (
        tc.tile_pool(name="persist", bufs=1) as P,
        tc.tile_pool(name="tmp", bufs=2) as T,
    ):
        aff_t = P.tile([128, len(_TAPS), 2, W], F32, tag="aff", name="aff")
        x_t = [P.tile([128, 2, WP], F32, tag=f"x{i}", name=f"x{i}")
               for i in range(2)]
        xs_t = {dr: P.tile([128, 2, WP], F32, tag=f"xs{dr}", name=f"xs{dr}")
                for dr in DRS}
        rn_t = P.tile([128, 2, W], F32, tag="rn", name="rn")
        msk_t = P.tile([128, 2, W], F32, tag="msk", name="msk")
        sz_t = P.tile([128, 2, W], F32, tag="sz", name="sz")

        # ---- x init ----
        for xt in list(x_t) + list(xs_t.values()):
            nc.gpsimd.memset(xt[:, :, :], 0.0)
        nc.sync.dma_start(x_t[0][0:PANEL, 0, 2:386], feat_d[0:PANEL, :])
        nc.sync.dma_start(x_t[0][0:PANEL, 1, 2:386], feat_d[PANEL:EXT, :])
        nc.sync.dma_start(msk_t[0:PANEL, 0, :], mask_d[0:PANEL, :])
        nc.sync.dma_start(msk_t[0:PANEL, 1, :], mask_d[PANEL:EXT, :])
        nc.vector.tensor_mul(x_t[0][0:PANEL, :, 2:386],
                             x_t[0][0:PANEL, :, 2:386], msk_t[0:PANEL, :, :])

        # ---- affinity precompute (grouped by dr so img tiles stay small) ----
        # img_dr[c][p, s, w] = raw img(ext row panel_base(s)+p+dr, col w-2),
        # pads filled with -10 by the host. tap j order must match _TAPS.
        with tc.tile_pool(name="imgpool", bufs=2) as IP:
            img0 = [IP.tile([128, 2, WP], F32, tag=f"img0_{c}", name=f"img0_{c}",
                            bufs=1) for c in range(C)]
            for c in range(C):
                for s in (0, 1):
                    base = 2 + s * PANEL  # slab row of ext row panel base
                    nc.sync.dma_start(img0[c][0:PANEL, s, :],
                                      img_d[c, base:base + PANEL, :])
            first = True
            for dr in (0, 2, -2, 1, -1):
                if dr == 0:
                    imgs = img0
                else:
                    imgs = [IP.tile([128, 2, WP], F32, tag=f"imgs_{c}",
                                    name=f"imgs_{c}") for c in range(C)]
                    for c in range(C):
                        for s in (0, 1):
                            base = 2 + s * PANEL + dr
                            nc.sync.dma_start(imgs[c][0:PANEL, s, :],
                                              img_d[c, base:base + PANEL, :])
                for dw in (-2, -1, 0, 1, 2):
                    if dr == 0 and dw == 0:
                        continue
                    j = _TAPS.index((dr, dw))
                    d = [T.tile([128, 2, W], F32, tag=f"d{c}", name=f"d{c}")
                         for c in range(C)]
                    for c in range(C):
                        nc.vector.tensor_sub(
                            d[c][0:PANEL],
                            imgs[c][0:PANEL, :, 2 + dw:386 + dw],
                            img0[c][0:PANEL, :, 2:386])
                        nc.scalar.square(d[c][0:PANEL], d[c][0:PANEL])
                    ssd = T.tile([128, 2, W], F32, tag="ssd", name="ssd")
                    nc.vector.tensor_add(ssd[0:PANEL], d[0][0:PANEL],
                                         d[1][0:PANEL])
                    nc.vector.tensor_add(ssd[0:PANEL], ssd[0:PANEL],
                                         d[2][0:PANEL])
                    nc.scalar.activation(aff_t[0:PANEL, j, :, :],
                                         ssd[0:PANEL], Exp, scale=NZ)
                    if first:
                        # +1.0 accounts for the center tap (aff = exp(0) = 1)
                        nc.vector.tensor_scalar_add(
                            sz_t[0:PANEL], aff_t[0:PANEL, j, :, :], 1.0)
                        first = False
                    else:
                        nc.vector.tensor_add(sz_t[0:PANEL], sz_t[0:PANEL],
                                             aff_t[0:PANEL, j, :, :])
        nc.vector.reciprocal(rn_t[0:PANEL], sz_t[0:PANEL])
        nc.vector.tensor_mul(rn_t[0:PANEL], rn_t[0:PANEL], msk_t[0:PANEL])

        # ---- 20 propagation iterations ----
        for t in range(NUM_ITER):
            cur, nxt = x_t[t % 2], x_t[(t + 1) % 2]
            # refresh row-shifted copies of cur (SBUF->SBUF DMA):
            #   xs[dr][p, s] = ext row (panel_base(s) + p + dr)
            nc.sync.dma_start(xs_t[1][0:PANEL - 1, :, :], cur[1:PANEL, :, :])
            nc.sync.dma_start(xs_t[1][PANEL - 1:PANEL, 0, :], cur[0:1, 1, :])
            nc.sync.dma_start(xs_t[2][0:PANEL - 2, :, :], cur[2:PANEL, :, :])
            nc.sync.dma_start(xs_t[2][PANEL - 2:PANEL, 0, :], cur[0:2, 1, :])
            nc.sync.dma_start(xs_t[-1][1:PANEL, :, :], cur[0:PANEL - 1, :, :])
            nc.sync.dma_start(xs_t[-1][0:1, 1, :], cur[PANEL - 1:PANEL, 0, :])
            nc.sync.dma_start(xs_t[-2][2:PANEL, :, :], cur[0:PANEL - 2, :, :])
            nc.sync.dma_start(xs_t[-2][0:2, 1, :], cur[PANEL - 2:PANEL, 0, :])
            # (xs[+1][115,1], xs[+2][114:116,1], xs[-1][0,0], xs[-2][0:2,0]
            #  stay zero from the one-time memset: cut/pad rows.)

            acc = nxt[0:PANEL, :, 2:386]
            for j, (dr, dw) in enumerate(_TAPS):
                src = cur if dr == 0 else xs_t[dr]
                xs = src[0:PANEL, :, 2 + dw:386 + dw]
                a = aff_t[0:PANEL, j, :, :]
                if j == 0:
                    nc.vector.tensor_mul(acc, a, xs)
                else:
                    tmp = T.tile([128, 2, W], F32, tag="acc_tmp",
                                 name="acc_tmp")
                    nc.vector.tensor_mul(tmp[0:PANEL], a, xs)
                    nc.vector.tensor_add(acc, acc, tmp[0:PANEL])
            nc.vector.tensor_add(acc, acc, cur[0:PANEL, :, 2:386])  # center
            nc.vector.tensor_mul(acc, acc, rn_t[0:PANEL])

        xf = x_t[NUM_ITER % 2]
        nc.sync.dma_start(out_d[0:PANEL, :], xf[0:PANEL, 0, 2:386])
        nc.sync.dma_start(out_d[PANEL:EXT, :], xf[0:PANEL, 1, 2:386])


def _program():
    if "nc" not in _CACHE:
        nc = bacc.Bacc("TRN2", target_bir_lowering=False, debug=False)
        img_d = nc.dram_tensor("img", [C, PB, WP], F32, kind="ExternalInput")
        feat_d = nc.dram_tensor("feat", [EXT, W], F32, kind="ExternalInput")
        mask_d = nc.dram_tensor("mask", [EXT, W], F32, kind="ExternalInput")
        out_d = nc.dram_tensor("out", [EXT, W], F32, kind="ExternalOutput")
        with tile.TileContext(nc) as tc:
            _body(tc, img_d.ap(), feat_d.ap(), mask_d.ap(), out_d.ap())
        nc.compile()
        _CACHE["nc"] = nc
    return _CACHE["nc"]


def kernel(img, feat, masked_box):
    img = np.asarray(img, np.float32)
    feat = np.asarray(feat, np.float32)
    mask = np.asarray(masked_box, np.float32)

    in_maps = []
    for core in range(8):
        b, half = core // 2, core % 2
        e0 = 0 if half == 0 else H - EXT
        slab = np.full((C, PB, WP), -10.0, np.float32)
        r0 = e0 - 2
        lo, hi = max(r0, 0), min(r0 + PB, H)
        slab[:, lo - r0:hi - r0, 2:386] = img[b, :, lo:hi, :]
        in_maps.append({
            "img": slab,
            "feat": np.ascontiguousarray(feat[b, e0:e0 + EXT, :]),
            "mask": np.ascontiguousarray(mask[b, e0:e0 + EXT, :]),
        })

    trace = os.environ.get("BASS_KERNEL_TRACE", "0") == "1"
    res = run_bass_kernel_spmd(_program(), in_maps, core_ids=list(range(8)),
                               trace=trace)
    _CACHE["last_results"] = res

    out = np.empty((B, H, W), np.float32)
    for core in range(8):
        b, half = core // 2, core % 2
        r = res.results[core]["out"]
        if half == 0:
            out[b, :HALF] = r[:HALF]
        else:
            out[b, HALF:] = r[HALO:]
    return out


# revision 18
# speedup vs baseline: 1.2965x; 1.2965x over previous
"""Trainium2 Bass kernel for nn_Local_APro: affinity-based local propagation.

Reference computation (per image):
  F = img + 10
  aff_k = exp(-||F(p+delta_k) - F(p)||^2 / zeta^2)   (5x5 window, zero pad)
  x0    = feat * mask
  repeat 20x:  x <- mask * (sum_k aff_k * x(p+delta_k)) / sum_k aff_k

Sharding: 8 shards = 4 images x 2 height-halves, 40-row halo, no cross-core
communication; contaminated halo rows are discarded at host gather.

This environment executes ~1 instruction per ~33us regardless of engine or
operand size, so the design minimizes INSTRUCTION COUNT:

 * Per-core rows live in two 128-row panels (ext rows 0..127 and 104..231).
   Each panel evolves all 128 of its rows; panels exchange 12-row halos only
   every 6 iterations (2 small DMAs x 3), which is exactly enough to keep
   each panel's owned rows (0..115 / 116..231) uncontaminated.
 * x is kept in "xrep" tiles [128p, 5 row-shift slabs, 2 panels, 388 cols]:
   slab j holds x shifted by dr = j-2 rows (partition-shifted planar
   SBUF->SBUF DMA copies; compute engines cannot read partition offsets that
   are not quadrant-aligned). 4 bulk DMAs per iteration.
 * The whole 25-tap weighted stencil is 2 instructions per panel:
   one tensor_tensor with a 4D overlapping-window access pattern
   ([p][w][dw][dr-slab], where the affinity tensor is stored with tap index
   k = dw*5+dr so its innermost 25 values are contiguous), and one
   tensor_reduce(axis=XY) that sums the 25 taps and writes the new x
   directly into the next xrep's center slab.
 * aff is pre-normalized (aff_n = aff * mask / sumz, center tap holds
   mask/sumz itself), so iterations have no divide/mask cost.
 * The +10 shift cancels inside every img difference except against zero
   padding, so the host fills pad cells with -10 in raw-image space and no
   on-device shift is needed.
"""

import os
import sys

import numpy as np

_REPO = "/opt/trn_rl_repo"
try:
    import concourse.bass  # noqa: F401
except Exception:
    if os.path.isdir(_REPO) and _REPO not in sys.path:
        sys.path.insert(0, _REPO)

import concourse.bacc as bacc
import concourse.mybir as mybir
from concourse.bass_types import AP
from concourse.bass_utils import run_bass_kernel_spmd

K = 5
ZETA = 0.15
NUM_ITER = 20
B, C, H, W = 4, 3, 384, 384
HALF = 192            # kept output rows per shard
HALO = 2 * NUM_ITER   # 40 contaminated rows next to the cut
EXT = HALF + HALO     # 232 rows computed per shard
PB = 236              # padded slab rows (2 + EXT + 2)
WP = W + 4            # padded width 388
PSHIFT = 104          # panel B base ext row (B owns rows 12..127 locally)
REFRESH = 6           # halo-exchange period in iterations
F32 = mybir.dt.float32

_CACHE: dict = {}


def _ap(t, off, dims):
    """Raw AP on sbuf tensor handle t: dims = [[step_elems, count], ...]."""
    return AP(t.ap().tensor, off, [list(d) for d in dims])


def _build():
    nc = bacc.Bacc("TRN2", target_bir_lowering=False, debug=False)
    img_d = nc.dram_tensor("img", [C, PB, WP], F32, kind="ExternalInput")
    feat_d = nc.dram_tensor("feat", [EXT, W], F32, kind="ExternalInput")
    mask_d = nc.dram_tensor("mask", [EXT, W], F32, kind="ExternalInput")
    out_d = nc.dram_tensor("out", [EXT, W], F32, kind="ExternalOutput")

    NZ = -1.0 / (ZETA * ZETA)
    Exp = mybir.ActivationFunctionType.Exp
    MUL = mybir.AluOpType.mult
    ADD = mybir.AluOpType.add
    SUB = mybir.AluOpType.subtract

    # tile pitches (elements per partition)
    XR = 5 * 2 * WP          # xrep [128, 5, 2, 388]
    AF = 2 * W * 25          # aff  [128, 2, 384, 25]
    PR = W * 25              # prod [128, 384, 25] (one panel at a time)
    RN = 2 * W               # rn/msk [128, 2, 384]
    IM = 6 * WP              # img  [128, 6(c,s), 388]
    DD = 6 * W * 5           # dd   [128, 6, 384, 5]
    SS = 2 * W * 5           # ssd  [128, 2, 384, 5]

    n_it = NUM_ITER * int(os.environ.get("BASS_KERNEL_REPEAT", "1"))

    with (
        nc.sbuf_tensor([128, 2, W, 25], F32) as aff,
        nc.sbuf_tensor([128, 5, 2, WP], F32) as xr0,
        nc.sbuf_tensor([128, 5, 2, WP], F32) as xr1,
        nc.sbuf_tensor([128, 2, W], F32) as rn,
        nc.sbuf_tensor([128, 2, W], F32) as msk,
        nc.semaphore() as dsem,
        nc.semaphore() as vsem,
        nc.semaphore() as ssem,
        nc.Block() as block,
    ):
        xr = [xr0, xr1]
        d = [0]   # dsem expected value
        v = [0]   # vsem
        s_ = [0]  # ssem

        # helper closures emit into per-engine streams; python tracks counts
        sync_prog = []
        vec_prog = []
        act_prog = []

        def dma(dst, src, note=""):
            def f(eng):
                eng.dma_start(dst, src).then_inc(dsem, 16)
            d[0] += 16
            sync_prog.append(f)

        def dwait(val):
            sync_prog.append(lambda eng, _v=val: eng.wait_ge(dsem, _v))

        def swait_v(val):
            sync_prog.append(lambda eng, _v=val: eng.wait_ge(vsem, _v))

        def vec(op, inc=True):
            def f(eng):
                ins = op()
                if inc:
                    ins.then_inc(vsem, 1)
            if inc:
                v[0] += 1
            vec_prog.append(f)

        def vwait_d(val):
            vec_prog.append(lambda eng, _v=val: eng.wait_ge(dsem, _v))

        def vwait_s(val):
            vec_prog.append(lambda eng, _v=val: eng.wait_ge(ssem, _v))

        def act(op):
            def f(eng):
                op().then_inc(ssem, 1)
            s_[0] += 1
            act_prog.append(f)

        def awaits_v(val):
            act_prog.append(lambda eng, _v=val: eng.wait_ge(vsem, _v))

        # ---------------- build the schedule ----------------
        # memsets (vector engine, before anything reads xrep)
        vec(lambda: nc.vector.memset(xr0.ap(), 0.0))
        vec(lambda: nc.vector.memset(xr1.ap(), 0.0))
        v_memset = v[0]

        # feat -> xr0 center slab interior; mask load
        # dst: partitions 0..127, slab 2, panel s, cols 2..386
        dst = _ap(xr0, 2 * 2 * WP + 2, [[XR, 128], [WP, 2], [1, W]])
        src = AP(feat_d.ap().tensor, 0, [[W, 128], [PSHIFT * W, 2], [1, W]])
        swait_v(v_memset)
        dma(dst, src, "feat")
        mdst = _ap(msk, 0, [[RN, 128], [W, 2], [1, W]])
        msrc = AP(mask_d.ap().tensor, 0, [[W, 128], [PSHIFT * W, 2], [1, W]])
        dma(mdst, msrc, "mask")
        d_init = d[0]

        # x0 = feat * mask (in place on xr0 slab 2 interior)
        x0ap = lambda t: _ap(t, 2 * 2 * WP + 2, [[XR, 128], [WP, 2], [1, W]])
        vwait_d(d_init)
        vec(lambda: nc.vector.tensor_tensor(
            out=x0ap(xr0), in0=x0ap(xr0), in1=msk.ap(), op=MUL))
        v_x0 = v[0]

        # ---- affinity precompute ----
        with (
            nc.sbuf_tensor([128, 6, WP], F32) as img0,
            nc.sbuf_tensor([128, 6, WP], F32) as imgd,
            nc.sbuf_tensor([128, 6, W, 5], F32) as dd,
            nc.sbuf_tensor([128, 2, W, 5], F32) as ssd,
        ):
            def img_load(t, dri):
                # slab row for (panel s, local p, shift dri) = s*104 + p + dri
                for s in range(2):
                    dst = _ap(t, s * WP, [[IM, 128], [2 * WP, 3], [1, WP]])
                    src = AP(img_d.ap().tensor, (dri + s * PSHIFT) * WP,
                             [[WP, 128], [PB * WP, 3], [1, WP]])
                    dma(dst, src, f"img{dri}s{s}")

            img_load(img0, 2)  # dr = 0
            d_img0 = d[0]

            groups = [0, 1, 3, 4, 2]  # dr_i order; dr = dr_i - 2; center last
            v_sub_prev_group = None
            s_exp = {}
            for gi, dri in enumerate(groups):
                if dri != 2:
                    if gi > 0:
                        # imgd reuse: wait until previous group's sub consumed it
                        assert v_sub_prev_group is not None
                        swait_v(v_sub_prev_group)
                    img_load(imgd, dri)
                    gimg = imgd
                else:
                    gimg = img0
                d_g = d[0]

                # dd = (imgd window) - (img0 center broadcast over dw)
                win = _ap(gimg, 0, [[IM, 128], [WP, 6], [1, W], [1, 5]])
                ctr = _ap(img0, 2, [[IM, 128], [WP, 6], [1, W], [0, 5]])
                ddf = _ap(dd, 0, [[DD, 128], [W * 5, 6], [1, W * 5]])
                vwait_d(d_g)
                if gi > 0:
                    # previous group's exp must read ssd before we rewrite it
                    vwait_s(s_exp[groups[gi - 1]])
                vec(lambda w_=win, c_=ctr: nc.vector.tensor_tensor(
                    out=_ap(dd, 0, [[DD, 128], [W * 5, 6], [5, W], [1, 5]]),
                    in0=w_, in1=c_, op=SUB))
                v_sub_prev_group = v[0]
                vec(lambda: nc.vector.tensor_tensor(
                    out=ddf, in0=ddf, in1=ddf, op=MUL))
                c_sl = lambda c: _ap(dd, c * 2 * W * 5,
                                     [[DD, 128], [W * 5, 2], [1, W * 5]])
                ssdf = _ap(ssd, 0, [[SS, 128], [W * 5, 2], [1, W * 5]])
                vec(lambda: nc.vector.tensor_tensor(
                    out=ssdf, in0=c_sl(0), in1=c_sl(1), op=ADD))
                vec(lambda: nc.vector.tensor_tensor(
                    out=ssdf, in0=ssdf, in1=c_sl(2), op=ADD))
                v_csum = v[0]

                # exp -> aff[..., k = dw*5 + dr_i]
                adst = _ap(aff, dri, [[AF, 128], [W * 25, 2], [25, W], [5, 5]])
                awaits_v(v_csum)
                act(lambda a_=adst: nc.scalar.activation(
                    out=a_, in_=_ap(ssd, 0,
                                    [[SS, 128], [W * 5, 2], [5, W], [1, 5]]),
                    func=Exp, scale=NZ))
                s_exp[dri] = s_[0]

            # sumz = reduce_k aff ; rn = mask / sumz ; aff *= rn (broadcast)
            s_all = s_[0]
            aflat = _ap(aff, 0, [[AF, 128], [W * 25, 2], [25, W], [1, 25]])
            vwait_s(s_all)
            vec(lambda: nc.vector.tensor_reduce(
                out=_ap(rn, 0, [[RN, 128], [W, 2], [1, W]]),
                in_=aflat, axis=mybir.AxisListType.X, op=ADD))
            vec(lambda: nc.vector.reciprocal(rn.ap(), rn.ap()))
            vec(lambda: nc.vector.tensor_tensor(
                out=rn.ap(), in0=rn.ap(), in1=msk.ap(), op=MUL))
            rbc = _ap(rn, 0, [[RN, 128], [W, 2], [1, W], [0, 25]])
            vec(lambda: nc.vector.tensor_tensor(
                out=aflat, in0=aflat, in1=rbc, op=MUL))
            # (center tap k=12 was computed as exp(0)=1 and normalized like
            #  the rest, so no special-casing is needed)

        # ---- bulk copy helper: fill shifted slabs of xrep `t` from slab 2
        def bulks(t):
            for dri in (0, 1, 3, 4):
                sh = dri - 2
                lo = max(0, -sh)          # dst partition range [lo, hi)
                hi = min(128, 128 - sh)
                ddst = _ap(t, dri * 2 * WP + lo * XR,
                           [[XR, hi - lo], [WP, 2], [1, WP]])
                dsrc = _ap(t, 2 * 2 * WP + (lo + sh) * XR,
                           [[XR, hi - lo], [WP, 2], [1, WP]])
                dma(ddst, dsrc, f"bulk{dri}")

        # initial bulks for xr0
        swait_v(v_x0)
        bulks(xr0)
        d_bulk = d[0]

        # ---- iterations ----
        with nc.sbuf_tensor([128, W, 25], F32) as prod:
            for t in range(n_it):
                xin, xout = xr[t % 2], xr[(t + 1) % 2]
                vwait_d(d_bulk)
                for s in range(2):
                    # prod = aff_n * window(xin)
                    in1 = _ap(xin, s * WP, [[XR, 128], [1, W], [1, 5],
                                            [2 * WP, 5]])
                    in0 = _ap(aff, s * W * 25, [[AF, 128], [25, W], [5, 5],
                                                [1, 5]])
                    po = _ap(prod, 0, [[PR, 128], [25, W], [5, 5], [1, 5]])
                    vec(lambda a_=in0, b_=in1, o_=po:
                        nc.vector.tensor_tensor(out=o_, in0=a_, in1=b_,
                                                op=MUL))
                    # x_new(panel s) = reduce taps -> xout slab 2 interior
                    ro = _ap(xout, 2 * 2 * WP + s * WP + 2,
                             [[XR, 128], [1, W]])
                    vec(lambda o_=ro: nc.vector.tensor_reduce(
                        out=o_,
                        in_=_ap(prod, 0, [[PR, 128], [25, W], [5, 5], [1, 5]]),
                        axis=mybir.AxisListType.XY, op=ADD))
                v_red = v[0]

                swait_v(v_red)
                if t != n_it - 1:
                    if (t + 1) % REFRESH == 0:
                        # halo exchange on xout slab 2:
                        #   A rows 116..128 <- B rows 12..24 (ext 116..127)
                        #   B rows 0..12    <- A rows 104..116 (ext 104..115)
                        ha = _ap(xout, 2 * 2 * WP + 116 * XR,
                                 [[XR, 12], [1, WP]])
                        hb = _ap(xout, 2 * 2 * WP + WP,
                                 [[XR, 12], [1, WP]])
                        dma(ha, _ap(xout, 2 * 2 * WP + WP + 12 * XR,
                                    [[XR, 12], [1, WP]]), "refA")
                        dma(hb, _ap(xout, 2 * 2 * WP + 104 * XR,
                                    [[XR, 12], [1, WP]]), "refB")
                        dwait(d[0])
                    bulks(xout)
                    d_bulk = d[0]

            # output: owned rows, one DMA per panel
            v_fin = v[0]
            xf = xr[n_it % 2]
            swait_v(v_fin)
            oa = AP(out_d.ap().tensor, 0, [[W, 116], [1, W]])
            sa = _ap(xf, 2 * 2 * WP + 2, [[XR, 116], [1, W]])
            dma(oa, sa, "outA")
            ob = AP(out_d.ap().tensor, 116 * W, [[W, 116], [1, W]])
            sb = _ap(xf, 2 * 2 * WP + WP + 2 + 12 * XR, [[XR, 116], [1, W]])
            dma(ob, sb, "outB")

        # ---------------- emit engine programs ----------------
        @block.sync
        def _(eng):
            for f in sync_prog:
                f(eng)

        @block.vector
        def _(eng):
            for f in vec_prog:
                f(eng)

        @block.scalar
        def _(eng):
            for f in act_prog:
                f(eng)

    nc.compile()
    return nc


def _program():
    if "nc" not in _CACHE:
        _CACHE["nc"] = _build()
    return _CACHE["nc"]


def kernel(img, feat, masked_box):
    img = np.asarray(img, np.float32)
    feat = np.asarray(feat, np.float32)
    mask = np.asarray(masked_box, np.float32)

    in_maps = []
    for core in range(8):
        b, half = core // 2, core % 2
        e0 = 0 if half == 0 else H - EXT
        slab = np.full((C, PB, WP), -10.0, np.float32)
        r0 = e0 - 2
        lo, hi = max(r0, 0), min(r0 + PB, H)
        slab[:, lo - r0:hi - r0, 2:386] = img[b, :, lo:hi, :]
        in_maps.append({
            "img": slab,
            "feat": np.ascontiguousarray(feat[b, e0:e0 + EXT, :]),
            "mask": np.ascontiguousarray(mask[b, e0:e0 + EXT, :]),
        })

    res = run_bass_kernel_spmd(_program(), in_maps, core_ids=list(range(8)))
    _CACHE["last_results"] = res

    out = np.empty((B, H, W), np.float32)
    for core in range(8):
        b, half = core // 2, core % 2
        r = res.results[core]["out"]
        if half == 0:
            out[b, :HALF] = r[:HALF]
        else:
            out[b, HALF:] = r[HALO:]
    return out
